# revision 15
# baseline (speedup 1.0000x reference)
"""Distributed causal MHA for Trainium2 (8 NeuronCores).

Problem: B=2, S=2048, D=1024, H=16, A=64 causal attention block.

Sharding: tensor-parallel over heads. Each core owns 2 heads end-to-end
(QKV projection + attention), then an AllToAll exchanges z from
head-sharded to sequence-sharded, and each core computes the output
projection for its 512-token shard (contraction over all 16 heads).
Host concatenates the 8 token shards. No reduction on host.

Layouts (per core):
  xT   [D, T=4096]    residual transposed, tokens b-major (bf16)
  wq/wk/wv [D, 128]   2 local heads packed (h*64+a) (bf16)
  woR  [8, 128, D]    full Wo as (pair j, (h,a), d) (bf16)
  qT/kT [128, T]      (h*64+a) on partitions
  v    [128p=t, 32tc, 2h, 65]  v augmented with ones column (denominator trick)
  zT   [128, T]       attention out, head-packed
  out  [D, 512]       output-projection result for this core's token shard (f32)
"""

import numpy as np
import ml_dtypes

import concourse.bass as bass
import concourse.mybir as mybir
import concourse.tile as tile
from concourse import bacc
from concourse.bass import ts, ds
from concourse.bass_utils import run_bass_kernel_spmd

AF = mybir.ActivationFunctionType
F32 = mybir.dt.float32
BF16 = mybir.dt.bfloat16

B, S, D, H, A = 2, 2048, 1024, 16, 64
NCORES = 8
HPC = H // NCORES          # 2 heads per core
T = B * S                  # 4096 tokens
P = 128
DK = D // P                # 8 contraction chunks
NT = 512                   # free-dim tile for projections / attention
TSH = T // NCORES          # 512 tokens per core shard (output)
SCALE = 1.0 / np.sqrt(A)
NEG = -1.0e9

_CACHE = {}


def _build(collective=True, repeat=1):
    nc = bacc.Bacc("TRN2", target_bir_lowering=False, debug=False,
                   num_devices=NCORES)
    xT = nc.dram_tensor("xT", [D, T], BF16, kind="ExternalInput")
    wq = nc.dram_tensor("wq", [D, 2 * A], BF16, kind="ExternalInput")
    wk = nc.dram_tensor("wk", [D, 2 * A], BF16, kind="ExternalInput")
    wv = nc.dram_tensor("wv", [D, 2 * A], BF16, kind="ExternalInput")
    woR = nc.dram_tensor("woR", [NCORES, 2 * A, D], BF16, kind="ExternalInput")
    bqr = nc.dram_tensor("bqr", [1, 2 * A], BF16, kind="ExternalInput")
    bkr = nc.dram_tensor("bkr", [1, 2 * A], BF16, kind="ExternalInput")
    bvd = nc.dram_tensor("bvd", [1, 2 * A], F32, kind="ExternalInput")
    bod = nc.dram_tensor("bod", [1, D], BF16, kind="ExternalInput")
    mkd = nc.dram_tensor("mkd", [P, P], BF16, kind="ExternalInput")
    out = nc.dram_tensor("out", [D, TSH], F32, kind="ExternalOutput")

    with tile.TileContext(nc) as tc:
        with tc.tile_pool(name="big", bufs=1) as big, \
             tc.tile_pool(name="work", bufs=3) as work, \
             tc.tile_pool(name="mm_ps", bufs=2, space="PSUM") as mm_ps, \
             tc.tile_pool(name="sc_ps", bufs=3, space="PSUM") as sc_ps, \
             tc.tile_pool(name="z_ps", bufs=2, space="PSUM") as z_ps, \
             tc.tile_pool(name="dram", bufs=1, space="DRAM") as dram:

            # persistent tiles allocated once (reused across repeats)
            x_sb = big.tile([P, DK, T], BF16)
            wq_sb = big.tile([P, DK, 2 * A], BF16)
            wk_sb = big.tile([P, DK, 2 * A], BF16)
            wv_sb = big.tile([P, DK, 2 * A], BF16)
            wo_sb = big.tile([P, NCORES, DK, P], BF16)
            bqr_sb = big.tile([1, 2 * A], BF16)
            bkr_sb = big.tile([1, 2 * A], BF16)
            bv_sb = big.tile([1, 2 * A], F32)
            bo_sb = big.tile([1, D], BF16)
            mask_sb = big.tile([P, P], BF16)
            onesr = big.tile([1, NT], BF16)
            nc.any.memset(onesr[:], 1.0)
            onesc = big.tile([1, P], F32)
            nc.any.memset(onesc[:], 1.0)
            qT_sb = big.tile([P, T], BF16)     # (h*64+a, token)
            kT_sb = big.tile([P, T], BF16)
            v_sb = big.tile([P, T // P, HPC, A + 1], BF16)
            nc.any.memset(v_sb[:, :, :, A], 1.0)
            zT_sb = big.tile([P, T], BF16)
            SH = S // NCORES  # 256
            WA = 192          # 3/4 of batch 1, exchanged early
            WB = 64           # last s-tile sliver, exchanged at the end
            _szs = (SH, WA, WB)
            a2a_in = [dram.tile([NCORES, P, _szs[k]], BF16,
                                name=f"a2a_in{k}") for k in range(3)]
            a2a_out = [dram.tile([NCORES, P, _szs[k]], BF16,
                                 name=f"a2a_out{k}") for k in range(3)]
            zr_sb = big.tile([P, B, NCORES, SH], BF16)

            for _rep in range(repeat):
                # ---- load inputs to SBUF: tiny tensors, then the weights
                # and x chunks in first-use order so the prologue starts early
                nc.sync.dma_start(bqr_sb[:], bqr[:])
                nc.sync.dma_start(bkr_sb[:], bkr[:])
                nc.sync.dma_start(bv_sb[:], bvd[:])
                nc.sync.dma_start(bo_sb[:], bod[:])
                nc.sync.dma_start(mask_sb[:], mkd[:])
                nc.sync.dma_start(wq_sb[:], wq.ap().rearrange("(ko p) m -> p ko m", p=P))
                nc.sync.dma_start(wv_sb[:], wv.ap().rearrange("(ko p) m -> p ko m", p=P))
                for ko in range(DK):
                    nc.sync.dma_start(x_sb[:, ko, ts(0, T // 8)],
                                      xT.ap()[ts(ko, P), ts(0, T // 8)])
                nc.sync.dma_start(wk_sb[:], wk.ap().rearrange("(ko p) m -> p ko m", p=P))
                for tq in range(1, 8):
                    for ko in range(DK):
                        nc.sync.dma_start(x_sb[:, ko, ts(tq, T // 8)],
                                          xT.ap()[ts(ko, P), ts(tq, T // 8)])
                nc.sync.dma_start(
                    wo_sb[:], woR.ap().rearrange("j p (mo mi) -> p j mo mi", mi=P))


                # broadcast bv across partitions once (outer product w/ ones)
                bvb_sb = big.tile([P, HPC, A], F32, name="bvb")
                _bps = mm_ps.tile([P, NT], F32, tag="mm", name="bvb_ps")
                nc.tensor.matmul(_bps[:, 0:2 * A], onesc[:], bv_sb[:],
                                 start=True, stop=True)
                nc.vector.tensor_copy(out=bvb_sb[:], in_=_bps[:, 0:2 * A])

                # ---- projection: fused q/k chains + interleaved v pairs ----
                # The v matmuls (N=128) are ldweights-bound standalone; pairing
                # each q/k ko-matmul (N=512) with two v ko-matmuls hides the
                # v stationary loads (lds 321ns vs matmuls 319ns per step).
                SPB = S // NT   # 4 q/k tiles per batch

                def fused_unit_steps(dst, w_sb, brow, nt, tc0, on_act):
                    box = {}

                    def pre():
                        box["ps"] = mm_ps.tile([P, NT], F32, tag="mm",
                                               name="pqk")
                        box["pv"] = mm_ps.tile([P, 2, P], F32, tag="pv",
                                               bufs=1, name="pv")
                        # seed q/k PSUM with the bias row via a K=1 matmul
                        nc.tensor.matmul(box["ps"][:], brow[:], onesr[:],
                                         start=True, stop=False)

                    def ko_step(ko):
                        ps, pv = box["ps"], box["pv"]
                        for c in range(2):
                            nc.tensor.matmul(
                                pv[:, c, :], x_sb[:, ko, ts(tc0 + c, P)],
                                wv_sb[:, ko, :],
                                start=(ko == 0 and c == 0),
                                stop=(ko == DK - 1))
                        nc.tensor.matmul(ps[:], w_sb[:, ko, :],
                                         x_sb[:, ko, ts(nt, NT)],
                                         start=False, stop=(ko == DK - 1))

                    def fin():
                        ps, pv = box["ps"], box["pv"]
                        if on_act:
                            nc.scalar.copy(dst[:, ts(nt, NT)], ps[:])
                        else:
                            nc.vector.tensor_copy(out=dst[:, ts(nt, NT)],
                                                  in_=ps[:])
                        for c in range(2):
                            nc.vector.tensor_tensor(
                                out=v_sb[:, tc0 + c, :, 0:A],
                                in0=pv[:, c, :].rearrange("p (h a) -> p h a",
                                                          h=HPC),
                                in1=bvb_sb[:], op=mybir.AluOpType.add)

                    return ([pre] + [(lambda ko=ko: ko_step(ko))
                                     for ko in range(DK)] + [fin])

                # batch-0 projections inline (prologue)
                for nt in range(SPB):
                    for f in fused_unit_steps(qT_sb, wq_sb, bqr_sb, nt,
                                              4 * nt, True):
                        f()
                    for f in fused_unit_steps(kT_sb, wk_sb, bkr_sb, nt,
                                              4 * nt + 2, True):
                        f()

                # batch-1 projections run as background PE work inside
                # attention, micro-stepped to avoid long PE stalls
                bg = []
                for nt in range(SPB, 2 * SPB):
                    bg += fused_unit_steps(qT_sb, wq_sb, bqr_sb, nt,
                                           4 * nt, False)
                    bg += fused_unit_steps(kT_sb, wk_sb, bkr_sb, nt,
                                           4 * nt + 2, False)

                # ---- attention: flat pipelined stream, heads paired on PE ----
                LOOK = 1  # lookahead in paired-score units (2 psum tiles each)
                stream = []
                for b in range(B):
                    for st in range(S // NT):
                        nblk = 4 * st + 4
                        for tb in range(nblk):
                            stream.append((b, st, tb, nblk))

                def issue_scores(i):
                    b, st, tb, nblk = stream[i]
                    base = b * S
                    m = tb - 4 * st
                    w = 128 * m if m > 0 else 0
                    sps = []
                    for h in range(HPC):
                        hs = ds(h * A, A)
                        sp = sc_ps.tile([P, NT], F32, tag="sc", name=f"sp{h}")
                        # h=1 operands live at base partition 64 -> row-group 64;
                        # the two K=64 matmuls execute concurrently on the PE.
                        # Columns < w of a diagonal tile are fully masked: skip.
                        nc.tensor.matmul(
                            sp[:, w:],
                            kT_sb[hs, base + 128 * tb:base + 128 * (tb + 1)],
                            qT_sb[hs, base + NT * st + w:base + NT * (st + 1)],
                            start=True, stop=True)
                        sps.append(sp)
                    return sps


                def emit_a2a(k, width, src_off, dst_off):
                    # exchange `width`-token shards starting at token src_off;
                    # received slab lands at zr column dst_off
                    c = 0 if k == 0 else 1
                    for j in range(NCORES):
                        nc.sync.dma_start(
                            a2a_in[k][j],
                            zT_sb[:, ds(src_off + width * j, width)])
                    if collective:
                        nc.gpsimd.collective_compute(
                            "AllToAll", mybir.AluOpType.bypass,
                            replica_groups=[list(range(NCORES))],
                            ins=[a2a_in[k].opt()], outs=[a2a_out[k].opt()])
                    else:
                        nc.sync.dma_start(a2a_out[k][:], a2a_in[k][:])
                    for j in range(NCORES):
                        nc.sync.dma_start(
                            zr_sb[:, c, j, dst_off:dst_off + width],
                            a2a_out[k][j])

                def outproj_unit(c, mo, w0=0, wlen=None):
                    wlen = SH if wlen is None else wlen
                    ps = mm_ps.tile([P, NT], F32, tag="mm", name="po")
                    po = ps[:, 0:wlen]
                    # seed PSUM with the bias via a K=1 matmul, then accumulate
                    nc.tensor.matmul(po, bo_sb[:, ts(mo, P)], onesr[:, 0:wlen],
                                     start=True, stop=False)
                    for j in range(NCORES):
                        nc.tensor.matmul(po, wo_sb[:, j, mo, :],
                                         zr_sb[:, c, j, w0:w0 + wlen],
                                         start=False, stop=(j == NCORES - 1))
                    osb = work.tile([P, SH], F32, tag="o", bufs=8)
                    nc.vector.tensor_copy(out=osb[:, 0:wlen], in_=po)
                    nc.sync.dma_start(
                        out.ap()[ts(mo, P), ds(c * SH + w0, wlen)],
                        osb[:, 0:wlen])

                bg2 = []   # out-proj c=0 units, gated to late b=1 attention
                BG2_GATE = int(len(stream) * 0.78)
                zps = [None, None]
                sq = [issue_scores(i) for i in range(min(LOOK, len(stream)))]
                for i, (b, st, tb, nblk) in enumerate(stream):
                    if i + LOOK < len(stream):
                        sq.append(issue_scores(i + LOOK))
                    if bg:
                        bg.pop(0)()
                        if bg:
                            bg.pop(0)()
                    elif bg2 and i >= BG2_GATE:
                        bg2.pop(0)()
                    sps = sq.pop(0)
                    m = tb - 4 * st
                    # columns < 128m of a diagonal tile are fully masked: skip
                    # them in exp and in the z accumulation; only the 128-wide
                    # diagonal block needs the triangular mask multiply.
                    w = 128 * m if m > 0 else 0
                    for h in range(HPC):
                        if tb == 0:
                            zps[h] = z_ps.tile([A + 1, NT], F32, tag="z",
                                               name=f"zp{h}")
                        p_sb = work.tile([P, NT], BF16, tag="p", bufs=6)
                        nc.scalar.activation(p_sb[:, w:], sps[h][:, w:], AF.Exp,
                                             scale=SCALE)
                        if m >= 0:
                            nc.vector.tensor_tensor(
                                out=p_sb[:, ds(w, P)], in0=p_sb[:, ds(w, P)],
                                in1=mask_sb[:], op=mybir.AluOpType.mult)
                        nc.tensor.matmul(
                            zps[h][:, w:], v_sb[:, b * (S // P) + tb, h, :],
                            p_sb[:, w:],
                            start=(tb == 0), stop=(tb == nblk - 1))
                    if tb == nblk - 1:
                        base = b * S
                        for h in range(HPC):
                            hs = ds(h * A, A)
                            den = work.tile([1, NT], F32, tag="den", bufs=4)
                            nc.vector.tensor_copy(out=den[:], in_=zps[h][A:A + 1, :])
                            rec = work.tile([1, NT], F32, tag="rec", bufs=4)
                            nc.vector.reciprocal_approx_fast(rec[:], den[:])
                            bc_sb = work.tile([A, NT], F32, tag="bc", bufs=4)
                            nc.gpsimd.partition_broadcast(bc_sb[:], rec[:],
                                                          channels=A)
                            nc.vector.tensor_tensor(
                                out=zT_sb[hs, base + NT * st:base + NT * (st + 1)],
                                in0=zps[h][0:A, :], in1=bc_sb[:],
                                op=mybir.AluOpType.mult)
                        if b == 1 and st == 2:
                            # first 3 s-tiles of batch 1 exchanged early
                            emit_a2a(1, WA, S, 0)
                            bg2.extend([
                                (lambda mo=mo: outproj_unit(1, mo, 0, WA))
                                for mo in range(DK)])
                        if st == S // NT - 1:
                            if b == 0:
                                emit_a2a(0, SH, 0, 0)
                                bg2.extend([
                                    (lambda mo=mo: outproj_unit(0, mo))
                                    for mo in range(DK)])
                            else:
                                emit_a2a(2, WB, S + 3 * (S // 4), WA)

                # ---- remaining output projection (c=1 + any leftovers) ----
                while bg:
                    bg.pop(0)()
                while bg2:
                    bg2.pop(0)()
                for mo in range(DK):
                    outproj_unit(1, mo, WA, WB)

    nc.compile()
    return nc


def _prep_inputs(residual, Wq, Wk, Wv, Wo, bq, bk, bv, bo):
    bf = ml_dtypes.bfloat16
    residual = np.asarray(residual, np.float32)
    xT = np.ascontiguousarray(residual.reshape(T, D).T).astype(bf)
    woR = np.ascontiguousarray(
        np.asarray(Wo, np.float32).reshape(NCORES, 2 * A, D)).astype(bf)
    boR = np.ascontiguousarray(np.asarray(bo, np.float32).reshape(1, D)).astype(bf)
    # triangular mask for the 128-wide diagonal block (same for every m)
    tt = np.arange(P)[:, None]
    ss = np.arange(P)[None, :]
    mk = np.where(tt <= ss, 1.0, 0.0).astype(bf)
    in_maps = []
    for i in range(NCORES):
        hsl = slice(HPC * i, HPC * (i + 1))
        wqi = np.ascontiguousarray(
            np.asarray(Wq, np.float32)[hsl].transpose(1, 0, 2).reshape(D, 2 * A)
        ).astype(bf)
        wki = np.ascontiguousarray(
            np.asarray(Wk, np.float32)[hsl].transpose(1, 0, 2).reshape(D, 2 * A)
        ).astype(bf)
        wvi = np.ascontiguousarray(
            np.asarray(Wv, np.float32)[hsl].transpose(1, 0, 2).reshape(D, 2 * A)
        ).astype(bf)
        in_maps.append({
            "xT": xT, "wq": wqi, "wk": wki, "wv": wvi, "woR": woR,
            "bqr": np.asarray(bq, np.float32)[hsl].reshape(1, 2 * A).astype(bf),
            "bkr": np.asarray(bk, np.float32)[hsl].reshape(1, 2 * A).astype(bf),
            "bvd": np.asarray(bv, np.float32)[hsl].reshape(1, 2 * A),
            "bod": boR, "mkd": mk,
        })
    return in_maps


def kernel(residual, Wq, Wk, Wv, Wo, bq, bk, bv, bo, _trace=False):
    if "nc" not in _CACHE:
        _CACHE["nc"] = _build()
    nc = _CACHE["nc"]
    in_maps = _prep_inputs(residual, Wq, Wk, Wv, Wo, bq, bk, bv, bo)
    res = run_bass_kernel_spmd(nc, in_maps, core_ids=list(range(NCORES)),
                               trace=_trace)
    _CACHE["last_result"] = res
    SH = S // NCORES
    WA, WB = 192, 64
    full = np.empty((B, S, D), np.float32)
    for i in range(NCORES):
        o = res.results[i]["out"]  # [D, 2*SH]
        full[0, SH * i:SH * (i + 1), :] = o[:, 0:SH].T
        full[1, WA * i:WA * (i + 1), :] = o[:, SH:SH + WA].T
        full[1, 3 * (S // 4) + WB * i:3 * (S // 4) + WB * (i + 1), :] = \
            o[:, SH + WA:].T
    return full

